# revision 28
# baseline (speedup 1.0000x reference)
"""Causal attention (no 1/sqrt(d) scaling), B=8, S=2048, D=64, fp32 in/out.

Sharding: data-parallel over batch - one batch element per NeuronCore (8 cores).

Per-core algorithm (S=2048, D=64):
  - All matmuls are bf16: on TRN2 fp32r matmuls run 2 cycles/row effective
    (measured 427ns/512-row at any p-state) while bf16 runs 1 cycle/row at
    1.2GHz and can reach 2.4GHz under sustained gap-free streams. bf16 QK
    costs rel-err ~7e-3 (vs 1.8e-3 fp32), well under the 2e-2 gate.
  - Host packs qT/kT bf16 into one [64, 4096] tensor, [q|k] segments laid in
    consumption order so 3 sequential DMAs feed the chunks just in time.
  - Scores computed transposed per (q-chunk c, k-block pair) into 2-bank
    PSUM strips [128, 1024], matmuls trimmed to the causal column range
    col_lo = max(0, 128j - 512c).  ScalarE exps only causal columns (17408
    activate-columns, the per-core floor: ~1.04ns/col) from PSUM into bf16
    SBUF pair tiles; full pairs use one 1024-col activate (per-instruction
    overhead dominates smaller splits).
  - Causal masking: one shared [128,128] lower-tri bf16 mask, tensor_mul on
    DVE over each of the 16 diagonal blocks after exp.
  - PV per q-block i: [128, 66] PSUM accumulates matmul(lhsT=exp block j,
    rhs=vx block j) over j<=i; vx col 64 = ones -> the same matmul chain
    accumulates the softmax denominator. DVE normalizes via fast reciprocal
    + tensor_scalar multiply into a staging tile.
  - Schedule: chunk c's PV blocks are emitted between the next chunk's first
    two QK pairs and its remaining pairs, so the 3-strip PSUM pipeline keeps
    feeding ScalarE across chunk boundaries; chunk 3 splits its output DMA
    so only PV(15) + a 32KB DMA trail the last exp.
  - Host un-permutes the [128, 16*64] staging layout back to [2048, 64].
"""

import numpy as np

S = 2048
D = 64
B = 8
P = 128
CH = 512            # q-chunk width
NBLK = S // P       # 16 k-blocks
W = 66              # v | ones | pad

CORR = False        # q-split correction matmuls (2x QK rows, better accuracy)

_CACHED = {}


def _build():
    import concourse.bass as bass
    import concourse.bacc as bacc
    import concourse.mybir as mybir
    import concourse.tile as tile

    f32 = mybir.dt.float32
    bf16 = mybir.dt.bfloat16
    Exp = mybir.ActivationFunctionType.Exp

    nc = bacc.Bacc("TRN2", target_bir_lowering=False, debug=False,
                   enable_asserts=False, num_devices=B)

    assert not CORR, "CORR not supported in the [2,3,1,0] packed layout"
    kq_d = nc.dram_tensor("kq", (64, 2 * S), bf16, kind="ExternalInput")
    vx_d = nc.dram_tensor("vx", (P, NBLK * W), bf16, kind="ExternalInput")
    mask_d = nc.dram_tensor("mask", (P, P), bf16, kind="ExternalInput")
    out_d = nc.dram_tensor("out", (P, NBLK * D), f32, kind="ExternalOutput")

    with tile.TileContext(nc) as tc:
        with (
            tc.tile_pool(name="const", bufs=1) as cpool,
            tc.tile_pool(name="exps", bufs=20) as epool,
            tc.tile_pool(name="small", bufs=4) as spool,
            tc.tile_pool(name="spsum", bufs=3, space=bass.MemorySpace.PSUM) as sppool,
            tc.tile_pool(name="opsum", bufs=2, space=bass.MemorySpace.PSUM) as oppool,
        ):
            kq_s = cpool.tile([64, 2 * S], bf16, tag="kq", name="kq_s")
            vx_s = cpool.tile([P, NBLK * W], bf16, tag="vx", name="vx_s")
            mask_s = cpool.tile([P, P], bf16, tag="mask", name="mask_s")
            ostage = cpool.tile([P, NBLK * D], f32, tag="ostage", name="ostage_s")
            scr_in = cpool.tile([P, 1], f32, tag="scr_in", name="scr_in")
            scr_out = cpool.tile([P, 1], f32, tag="scr_out", name="scr_out")

            # Input DMAs in consumption order (chunk 2 first); mask/vx on
            # other engine queues so their issue overlaps.
            nc.sync.dma_start(kq_s[:, 0:2 * CH], kq_d.ap()[:, 0:2 * CH])
            nc.scalar.dma_start(mask_s[:], mask_d.ap()[:])
            nc.gpsimd.dma_start(vx_s[:], vx_d.ap()[:])

            # Warm the Exp activation table during the DMA lead-in.
            nc.gpsimd.memset(scr_in[:], 0.0)
            nc.scalar.activation(scr_out[:], scr_in[:], Exp)
            nc.sync.dma_start(kq_s[:, 2 * CH:6 * CH], kq_d.ap()[:, 2 * CH:6 * CH])
            nc.sync.dma_start(kq_s[:, 6 * CH:8 * CH], kq_d.ap()[:, 6 * CH:8 * CH])

            # Packed kq column layout: 3 windows in consumption order, each
            # [q | k] for x-ranges [0,512), [512,1536), [1536,2048).
            _w0 = [0, 1024, 3072]
            _wx = [0, 512, 1536]
            _wn = [512, 1024, 512]

            def _seg(x):
                return 0 if x < 512 else 1 if x < 1536 else 2

            def pq_hi(x):
                s = _seg(x)
                return _w0[s] + (x - _wx[s])

            def pk(x):
                s = _seg(x)
                return _w0[s] + _wn[s] + (x - _wx[s])

            eb = {}

            spd = {}

            def qk_mms(c, p):
                """Score matmuls for k-blocks (2p, 2p+1) vs chunk c."""
                j0, j1 = 2 * p, 2 * p + 1
                sp = sppool.tile([P, 2 * CH], f32, tag="scores", name="scores")
                spd[(c, p)] = sp
                for t, j in enumerate((j0, j1)):
                    lo = max(0, P * j - CH * c)
                    kc = pk(j * P)
                    qh = pq_hi(c * CH + lo)
                    nc.tensor.matmul(
                        sp[:, t * CH + lo:(t + 1) * CH],
                        kq_s[:, kc:kc + P],
                        kq_s[:, qh:qh + CH - lo],
                        start=True, stop=True)

            def qk_exp(c, p):
                """Exp (+ diag mask) for the pair computed by qk_mms(c, p)."""
                j0, j1 = 2 * p, 2 * p + 1
                sp = spd[(c, p)]
                ebt = epool.tile([P, 2 * CH], bf16, tag="eb", name="eb")
                eb[(c, p)] = ebt
                if j1 < 4 * c:
                    nc.scalar.activation(ebt[:], sp[:], Exp)
                else:
                    for t, j in enumerate((j0, j1)):
                        lo = max(0, P * j - CH * c)
                        nc.scalar.activation(
                            ebt[:, t * CH + lo:(t + 1) * CH],
                            sp[:, t * CH + lo:(t + 1) * CH], Exp)
                for t, j in enumerate((j0, j1)):
                    ii = j - 4 * c
                    if 0 <= ii < 4:
                        col = t * CH + P * ii
                        nc.gpsimd.tensor_mul(
                            ebt[:, col:col + P], ebt[:, col:col + P], mask_s[:])

            def qk_pair(c, p):
                qk_mms(c, p)
                qk_exp(c, p)

            def pv_block(c, ii, raw=False):
                """PV accumulation + normalize for q-block i = 4c + ii."""
                i = 4 * c + ii
                op = oppool.tile([P, W], f32, tag="outp", name="outp")
                for j in range(i + 1):
                    ebt = eb[(c, j // 2)]
                    col = (j % 2) * CH + ii * P
                    nc.tensor.matmul(
                        op[:], ebt[:, col:col + P], vx_s[:, j * W:(j + 1) * W],
                        start=(j == 0), stop=(j == i))
                if raw:
                    return op
                rc = spool.tile([P, 1], f32, tag="recip", name="recip")
                nc.vector.reciprocal_approx_fast(rc[:], op[:, 64:65])
                nc.vector.tensor_scalar_mul(
                    ostage[:, i * D:(i + 1) * D], op[:, 0:D], rc[:])

            # Chunk order [0,1,2,3]: chunk c's PV blocks sit between the
            # next chunk's first two QK pairs and its remaining pairs so the
            # 3-strip pipeline keeps feeding ScalarE across boundaries;
            # chunk 3 splits its output DMA so only PV(15) + a 32KB DMA
            # trail the last exp.
            for c in range(4):
                if c == 0:
                    qk_pair(0, 0)
                    qk_pair(0, 1)
                if c < 3:
                    qk_pair(c + 1, 0)
                    qk_pair(c + 1, 1)
                    for ii in range(4):
                        pv_block(c, ii)
                    nc.sync.dma_start(
                        out_d.ap()[:, 4 * c * D:(4 * c + 4) * D],
                        ostage[:, 4 * c * D:(4 * c + 4) * D])
                    for p in range(2, 2 * (c + 1) + 2):
                        qk_pair(c + 1, p)
                else:
                    for ii in range(3):
                        pv_block(3, ii)
                    nc.sync.dma_start(
                        out_d.ap()[:, 12 * D:15 * D], ostage[:, 12 * D:15 * D])
                    pv_block(3, 3)
                    nc.sync.dma_start(
                        out_d.ap()[:, 15 * D:16 * D], ostage[:, 15 * D:16 * D])

    nc.compile()
    return nc


def get_nc():
    if "nc" not in _CACHED:
        _CACHED["nc"] = _build()
    return _CACHED["nc"]


def make_in_maps(q, k, v):
    import ml_dtypes
    bf16 = ml_dtypes.bfloat16

    q = np.asarray(q, dtype=np.float32)
    k = np.asarray(k, dtype=np.float32)
    v = np.asarray(v, dtype=np.float32)

    kl = np.arange(P)[:, None]
    ql = np.arange(P)[None, :]
    mask = (ql >= kl).astype(bf16)

    in_maps = []
    for b in range(B):
        vx = np.zeros((NBLK, P, W), dtype=bf16)
        vx[:, :, :D] = v[b].reshape(NBLK, P, D).astype(bf16)
        vx[:, :, D] = bf16(1.0)
        vx = np.ascontiguousarray(
            vx.transpose(1, 0, 2)).reshape(P, NBLK * W)
        kT = k[b].T.astype(bf16)
        qh = q[b].T.astype(bf16)
        kq = np.concatenate([
            qh[:, 0:512], kT[:, 0:512],
            qh[:, 512:1536], kT[:, 512:1536],
            qh[:, 1536:2048], kT[:, 1536:2048],
        ], axis=1)
        in_maps.append({
            "kq": np.ascontiguousarray(kq),
            "vx": vx,
            "mask": mask,
        })
    return in_maps


def kernel(q, k, v):
    from concourse.bass_utils import run_bass_kernel_spmd

    nc = get_nc()
    in_maps = make_in_maps(q, k, v)
    res = run_bass_kernel_spmd(nc, in_maps, core_ids=list(range(B)))
    _CACHED["last_results"] = res
    out = np.stack([
        res.results[b]["out"].reshape(P, NBLK, D).transpose(1, 0, 2)
        .reshape(S, D)
        for b in range(B)
    ], axis=0)
    return out.astype(np.float32)


# revision 29
# speedup vs baseline: 1.0030x; 1.0030x over previous
"""Causal attention (no 1/sqrt(d) scaling), B=8, S=2048, D=64, fp32 in/out.

Sharding: data-parallel over batch - one batch element per NeuronCore (8 cores).

Per-core algorithm (S=2048, D=64):
  - All matmuls are bf16: on TRN2 fp32r matmuls run 2 cycles/row effective
    (measured 427ns/512-row at any p-state) while bf16 runs 1 cycle/row at
    1.2GHz and can reach 2.4GHz under sustained gap-free streams. bf16 QK
    costs rel-err ~7e-3 (vs 1.8e-3 fp32), well under the 2e-2 gate.
  - Host packs qT/kT bf16 into one [64, 4096] tensor, [q|k] segments laid in
    consumption order so 3 sequential DMAs feed the chunks just in time.
  - Scores computed transposed per (q-chunk c, k-block pair) into 2-bank
    PSUM strips [128, 1024], matmuls trimmed to the causal column range
    col_lo = max(0, 128j - 512c).  ScalarE exps only causal columns (17408
    activate-columns, the per-core floor: ~1.04ns/col) from PSUM into bf16
    SBUF pair tiles; full pairs use one 1024-col activate (per-instruction
    overhead dominates smaller splits).
  - Causal masking: one shared [128,128] lower-tri bf16 mask, tensor_mul on
    DVE over each of the 16 diagonal blocks after exp.
  - PV per q-block i: [128, 66] PSUM accumulates matmul(lhsT=exp block j,
    rhs=vx block j) over j<=i; vx col 64 = ones -> the same matmul chain
    accumulates the softmax denominator. DVE normalizes via fast reciprocal
    + tensor_scalar multiply into a staging tile.
  - Schedule: chunk c's PV blocks are emitted between the next chunk's first
    two QK pairs and its remaining pairs, so the 3-strip PSUM pipeline keeps
    feeding ScalarE across chunk boundaries; chunk 3 splits its output DMA
    so only PV(15) + a 32KB DMA trail the last exp.
  - Host un-permutes the [128, 16*64] staging layout back to [2048, 64].
"""

import numpy as np

S = 2048
D = 64
B = 8
P = 128
CH = 512            # q-chunk width
NBLK = S // P       # 16 k-blocks
W = 66              # v | ones | pad

CORR = False        # q-split correction matmuls (2x QK rows, better accuracy)

_CACHED = {}


def _build():
    import concourse.bass as bass
    import concourse.bacc as bacc
    import concourse.mybir as mybir
    import concourse.tile as tile

    f32 = mybir.dt.float32
    bf16 = mybir.dt.bfloat16
    Exp = mybir.ActivationFunctionType.Exp

    nc = bacc.Bacc("TRN2", target_bir_lowering=False, debug=False,
                   enable_asserts=False, num_devices=B)

    assert not CORR, "CORR not supported in the [2,3,1,0] packed layout"
    kq_d = nc.dram_tensor("kq", (64, 2 * S), bf16, kind="ExternalInput")
    vx_d = nc.dram_tensor("vx", (P, NBLK * W), bf16, kind="ExternalInput")
    mask_d = nc.dram_tensor("mask", (P, P), bf16, kind="ExternalInput")
    out_d = nc.dram_tensor("out", (P, NBLK * D), f32, kind="ExternalOutput")

    with tile.TileContext(nc) as tc:
        with (
            tc.tile_pool(name="const", bufs=1) as cpool,
            tc.tile_pool(name="exps", bufs=20) as epool,
            tc.tile_pool(name="small", bufs=4) as spool,
            tc.tile_pool(name="spsum", bufs=3, space=bass.MemorySpace.PSUM) as sppool,
            tc.tile_pool(name="opsum", bufs=2, space=bass.MemorySpace.PSUM) as oppool,
        ):
            kq_s = cpool.tile([64, 2 * S], bf16, tag="kq", name="kq_s")
            vx_s = cpool.tile([P, NBLK * W], bf16, tag="vx", name="vx_s")
            mask_s = cpool.tile([P, P], bf16, tag="mask", name="mask_s")
            ostage = cpool.tile([P, NBLK * D], f32, tag="ostage", name="ostage_s")
            scr_in = cpool.tile([P, 1], f32, tag="scr_in", name="scr_in")
            scr_out = cpool.tile([P, 1], f32, tag="scr_out", name="scr_out")

            # Input DMAs in consumption order (chunk 2 first); mask/vx on
            # other engine queues so their issue overlaps.
            nc.sync.dma_start(kq_s[:, 0:2 * CH], kq_d.ap()[:, 0:2 * CH])
            nc.scalar.dma_start(mask_s[:], mask_d.ap()[:])
            nc.gpsimd.dma_start(vx_s[:], vx_d.ap()[:])

            # Warm the Exp activation table during the DMA lead-in.
            nc.gpsimd.memset(scr_in[:], 0.0)
            nc.scalar.activation(scr_out[:], scr_in[:], Exp)
            nc.sync.dma_start(kq_s[:, 2 * CH:6 * CH], kq_d.ap()[:, 2 * CH:6 * CH])
            nc.sync.dma_start(kq_s[:, 6 * CH:8 * CH], kq_d.ap()[:, 6 * CH:8 * CH])

            # Packed kq column layout: 3 windows in consumption order, each
            # [q | k] for x-ranges [0,512), [512,1536), [1536,2048).
            _w0 = [0, 1024, 3072]
            _wx = [0, 512, 1536]
            _wn = [512, 1024, 512]

            def _seg(x):
                return 0 if x < 512 else 1 if x < 1536 else 2

            def pq_hi(x):
                s = _seg(x)
                return _w0[s] + (x - _wx[s])

            def pk(x):
                s = _seg(x)
                return _w0[s] + _wn[s] + (x - _wx[s])

            eb = {}

            spd = {}

            def qk_mms(c, p):
                """Score matmuls for k-blocks (2p, 2p+1) vs chunk c."""
                j0, j1 = 2 * p, 2 * p + 1
                sp = sppool.tile([P, 2 * CH], f32, tag="scores", name="scores")
                spd[(c, p)] = sp
                for t, j in enumerate((j0, j1)):
                    lo = max(0, P * j - CH * c)
                    kc = pk(j * P)
                    qh = pq_hi(c * CH + lo)
                    nc.tensor.matmul(
                        sp[:, t * CH + lo:(t + 1) * CH],
                        kq_s[:, kc:kc + P],
                        kq_s[:, qh:qh + CH - lo],
                        start=True, stop=True)

            def qk_exp(c, p):
                """Exp (+ diag mask) for the pair computed by qk_mms(c, p)."""
                j0, j1 = 2 * p, 2 * p + 1
                sp = spd[(c, p)]
                ebt = epool.tile([P, 2 * CH], bf16, tag="eb", name="eb")
                eb[(c, p)] = ebt
                if j1 < 4 * c:
                    nc.scalar.activation(ebt[:], sp[:], Exp)
                else:
                    for t, j in enumerate((j0, j1)):
                        lo = max(0, P * j - CH * c)
                        nc.scalar.activation(
                            ebt[:, t * CH + lo:(t + 1) * CH],
                            sp[:, t * CH + lo:(t + 1) * CH], Exp)
                for t, j in enumerate((j0, j1)):
                    ii = j - 4 * c
                    if 0 <= ii < 4:
                        col = t * CH + P * ii
                        nc.vector.tensor_mul(
                            ebt[:, col:col + P], ebt[:, col:col + P], mask_s[:])

            def qk_pair(c, p):
                qk_mms(c, p)
                qk_exp(c, p)

            def pv_block(c, ii, raw=False):
                """PV accumulation + normalize for q-block i = 4c + ii."""
                i = 4 * c + ii
                op = oppool.tile([P, W], f32, tag="outp", name="outp")
                for j in range(i + 1):
                    ebt = eb[(c, j // 2)]
                    col = (j % 2) * CH + ii * P
                    nc.tensor.matmul(
                        op[:], ebt[:, col:col + P], vx_s[:, j * W:(j + 1) * W],
                        start=(j == 0), stop=(j == i))
                if raw:
                    return op
                rc = spool.tile([P, 1], f32, tag="recip", name="recip")
                nc.vector.reciprocal_approx_fast(rc[:], op[:, 64:65])
                nc.vector.tensor_scalar_mul(
                    ostage[:, i * D:(i + 1) * D], op[:, 0:D], rc[:])

            # Chunk order [0,1,2,3]: chunk c's PV blocks sit between the
            # next chunk's first two QK pairs and its remaining pairs so the
            # 3-strip pipeline keeps feeding ScalarE across boundaries;
            # chunk 3 splits its output DMA so only PV(15) + a 32KB DMA
            # trail the last exp.
            for c in range(4):
                if c == 0:
                    qk_pair(0, 0)
                    qk_pair(0, 1)
                if c < 3:
                    qk_pair(c + 1, 0)
                    qk_pair(c + 1, 1)
                    for ii in range(4):
                        pv_block(c, ii)
                    nc.sync.dma_start(
                        out_d.ap()[:, 4 * c * D:(4 * c + 4) * D],
                        ostage[:, 4 * c * D:(4 * c + 4) * D])
                    for p in range(2, 2 * (c + 1) + 2):
                        qk_pair(c + 1, p)
                else:
                    for ii in range(3):
                        pv_block(3, ii)
                    nc.sync.dma_start(
                        out_d.ap()[:, 12 * D:15 * D], ostage[:, 12 * D:15 * D])
                    pv_block(3, 3)
                    nc.sync.dma_start(
                        out_d.ap()[:, 15 * D:16 * D], ostage[:, 15 * D:16 * D])

    nc.compile()
    return nc


def get_nc():
    if "nc" not in _CACHED:
        _CACHED["nc"] = _build()
    return _CACHED["nc"]


def make_in_maps(q, k, v):
    import ml_dtypes
    bf16 = ml_dtypes.bfloat16

    q = np.asarray(q, dtype=np.float32)
    k = np.asarray(k, dtype=np.float32)
    v = np.asarray(v, dtype=np.float32)

    kl = np.arange(P)[:, None]
    ql = np.arange(P)[None, :]
    mask = (ql >= kl).astype(bf16)

    in_maps = []
    for b in range(B):
        vx = np.zeros((NBLK, P, W), dtype=bf16)
        vx[:, :, :D] = v[b].reshape(NBLK, P, D).astype(bf16)
        vx[:, :, D] = bf16(1.0)
        vx = np.ascontiguousarray(
            vx.transpose(1, 0, 2)).reshape(P, NBLK * W)
        kT = k[b].T.astype(bf16)
        qh = q[b].T.astype(bf16)
        kq = np.concatenate([
            qh[:, 0:512], kT[:, 0:512],
            qh[:, 512:1536], kT[:, 512:1536],
            qh[:, 1536:2048], kT[:, 1536:2048],
        ], axis=1)
        in_maps.append({
            "kq": np.ascontiguousarray(kq),
            "vx": vx,
            "mask": mask,
        })
    return in_maps


def kernel(q, k, v):
    from concourse.bass_utils import run_bass_kernel_spmd

    nc = get_nc()
    in_maps = make_in_maps(q, k, v)
    res = run_bass_kernel_spmd(nc, in_maps, core_ids=list(range(B)))
    _CACHED["last_results"] = res
    out = np.stack([
        res.results[b]["out"].reshape(P, NBLK, D).transpose(1, 0, 2)
        .reshape(S, D)
        for b in range(B)
    ], axis=0)
    return out.astype(np.float32)
